# revision 41
# baseline (speedup 1.0000x reference)
# Trainium2 Bass kernel for a 2-layer edge-featured GAT (PyG GATConv style)
# with global mean pool, distributed over 8 NeuronCores.
#
# Sharding: edges are partitioned by destination node range (dst-shard); each
# core owns N/8 consecutive dst nodes and all edges pointing into them, so the
# segment softmax / aggregation is core-local.  Node features are computed
# replicated.  Layer-1 output is exchanged with an AllGather; the graph pool
# uses an AllReduce of per-core partial (sum, count) tables.
#
# Per-core algorithm (per layer):
#   h_ext = x @ [W | A_src | A_dst]      (A_* = W contracted with att vectors)
#   gather h_ext rows by edge src (dma_gather, spread over 4 SWDGE queues);
#   a_dst reaches edges via a per-block matmul against the transposed
#   selection matrix (no dst-side gather at all)
#   alpha = leaky_relu(a_src[src] + a_dst[dst] + ea @ B, 0.2); ex = exp(alpha)
#   (softmax max-subtraction is skipped: it cancels exactly in the ratio and
#    alpha is bounded by the data scale, far from fp32 overflow)
#   per 128-dst block: agg = sum_chunks S_chunk^T @ [ex*h | ex]   (PE matmuls,
#    S = one-hot dst selection built by a DVE compare against an iota)
#   out_row = elu(agg[:, :C] / agg[:, C:] + bias)

import numpy as np
import ml_dtypes

BF16 = ml_dtypes.bfloat16

# ---------------- problem constants (self-contained; must match grader) -----
N, E, NODE_IN, EDGE_IN, HIDDEN, HEADS, OUT_DIM, NUM_GRAPHS = (
    20000, 640000, 128, 16, 64, 4, 64, 64)
NCORES = 8
P = 128


class Cfg:
    def __init__(self, n=N, e=E, node_in=NODE_IN, edge_in=EDGE_IN, hidden=HIDDEN,
                 heads=HEADS, out_dim=OUT_DIM, num_graphs=NUM_GRAPHS, ncores=NCORES):
        self.N, self.E, self.NODE_IN, self.EDGE_IN = n, e, node_in, edge_in
        self.HIDDEN, self.HEADS, self.OUT_DIM, self.NUM_GRAPHS = hidden, heads, out_dim, num_graphs
        self.NCORES = ncores
        assert n % ncores == 0
        self.NLOC = n // ncores
        self.NBLK = (self.NLOC + P - 1) // P
        self.C1 = heads * hidden                      # 256
        self.ROW1 = self.C1 + 8                       # h1 bf16 + a_s (4 f32 = 8 bf16 slots)
        # round ROW1 (bf16 elements) so row bytes % 256 == 0
        self.ROW1 = ((self.ROW1 * 2 + 255) // 256) * 256 // 2       # 384
        self.ROW2 = ((self.OUT_DIM + 4 + 127) // 128) * 128         # bf16 elems, 256B -> 128
        self.KC2 = (self.C1 + P - 1) // P             # k-chunks for layer-2 matmul


# ---------------- host-side sharding prep (integer index work only) ---------
def _prep(cfg, edge_index, edge_attr, batch):
    n, e = cfg.N, cfg.E
    src = np.concatenate([np.asarray(edge_index[0]), np.arange(n)]).astype(np.int64)
    dst = np.concatenate([np.asarray(edge_index[1]), np.arange(n)]).astype(np.int64)
    ea = np.asarray(edge_attr, dtype=np.float32)
    mean_ea = ea.mean(axis=0, keepdims=True)
    ea_all = np.concatenate([ea, np.broadcast_to(mean_ea, (n, cfg.EDGE_IN))], axis=0)

    core = dst // cfg.NLOC
    dloc = dst - core * cfg.NLOC
    blk = dloc // P

    # max chunks per (core, block)
    M = 1
    buckets = {}
    for k in range(cfg.NCORES):
        mk = core == k
        for b in range(cfg.NBLK):
            sel = np.nonzero(mk & (blk == b))[0]
            buckets[(k, b)] = sel
            M = max(M, (len(sel) + P - 1) // P)
    EB = M * P
    EPAD = cfg.NBLK * EB
    SLOTS = EPAD // 16

    per_core = []
    for k in range(cfg.NCORES):
        srcp = np.zeros(EPAD, np.int16)
        dstp = np.zeros(EPAD, np.int16)
        dmod = np.full(EPAD, -1.0, np.float32)
        eaT = np.zeros((cfg.EDGE_IN, EPAD), np.float32)
        for b in range(cfg.NBLK):
            sel = buckets[(k, b)]
            o = b * EB
            nn = len(sel)
            srcp[o:o + nn] = src[sel]
            dstp[o:o + nn] = dst[sel]
            dmod[o:o + nn] = (dloc[sel] - b * P).astype(np.float32)
            eaT[:, o:o + nn] = ea_all[sel].T
        # wrapped idx layout: idx j -> partition j%16, slot j//16; replicated x8
        srcw = np.tile(srcp.reshape(-1, 16).T, (8, 1)).copy()     # [128, SLOTS]
        dstw = np.tile(dstp.reshape(-1, 16).T, (8, 1)).copy()
        # chunk-major layout: [p, b*M+c] = edge (b, c, p) — for indirect DMA
        src32 = srcp.reshape(cfg.NBLK * M, P).T.astype(np.int32).copy()
        dst32 = dstp.reshape(cfg.NBLK * M, P).T.astype(np.int32).copy()
        dmod_hm = dmod.reshape(cfg.NBLK * M, P).T.astype(BF16).copy()   # [128, NBLK*M]
        # batch ids of this core's nodes, [128, NBLK], pad -1
        bw = np.full((P, cfg.NBLK), -1.0, np.float32)
        bvals = np.asarray(batch[k * cfg.NLOC:(k + 1) * cfg.NLOC], dtype=np.float32)
        for b in range(cfg.NBLK):
            seg = bvals[b * P:(b + 1) * P]
            bw[:len(seg), b] = seg
        per_core.append(dict(
            srcw=srcw, dstw=dstw, src32=src32, dst32=dst32, dmod=dmod_hm,
            eaT=eaT.astype(BF16), batchw=bw.astype(BF16)))
    return per_core, M, SLOTS, EPAD


# ---------------- device program ------------------------------------------
def _build(cfg, M):
    import concourse.bass as bass
    import concourse.mybir as mybir
    import concourse.tile as tile
    from concourse import bacc
    from concourse.masks import make_identity

    fp32 = mybir.dt.float32
    bf16 = mybir.dt.bfloat16
    i16 = mybir.dt.int16
    AF = mybir.ActivationFunctionType
    OP = mybir.AluOpType

    H, C, OD = cfg.HEADS, cfg.HIDDEN, cfg.OUT_DIM
    C1 = cfg.C1
    NBLK, NLOC, NI, EI = cfg.NBLK, cfg.NLOC, cfg.NODE_IN, cfg.EDGE_IN
    EB = M * P
    EPAD = NBLK * EB
    SLOTS = EPAD // 16
    NT = (cfg.N + P - 1) // P          # global node tiles
    G = cfg.NUM_GRAPHS
    ROW1, ROW2, KC2 = cfg.ROW1, cfg.ROW2, cfg.KC2

    import os as _osq
    _NSWQ = int(_osq.environ.get("K_QN", "4"))
    nc = bacc.Bacc("TRN2", target_bir_lowering=False, debug=False,
                   enable_asserts=False, num_devices=cfg.NCORES,
                   num_swdge_queues=_NSWQ)
    groups = [list(range(cfg.NCORES))]

    # -------- external inputs --------
    xT = nc.dram_tensor("xT", [NI, cfg.N], fp32, kind="ExternalInput")
    srcw_d = nc.dram_tensor("srcw", [P, SLOTS], i16, kind="ExternalInput")
    xTloc_d = nc.dram_tensor("xTloc", [NI, NBLK * P], fp32, kind="ExternalInput")
    dmod_d = nc.dram_tensor("dmod", [P, NBLK * M], bf16, kind="ExternalInput")
    eaT_d = nc.dram_tensor("eaT", [EI, EPAD], bf16, kind="ExternalInput")
    batchw_d = nc.dram_tensor("batchw", [P, NBLK], bf16, kind="ExternalInput")
    W1_d = nc.dram_tensor("W1", [NI, C1], fp32, kind="ExternalInput")
    as1_d = nc.dram_tensor("as1", [1, C1], fp32, kind="ExternalInput")
    ad1_d = nc.dram_tensor("ad1", [1, C1], fp32, kind="ExternalInput")
    le1_d = nc.dram_tensor("le1", [EI, C1], fp32, kind="ExternalInput")
    ae1_d = nc.dram_tensor("ae1", [1, C1], fp32, kind="ExternalInput")
    b1_d = nc.dram_tensor("b1", [1, C1], fp32, kind="ExternalInput")
    W2_d = nc.dram_tensor("W2", [P, KC2 * OD], fp32, kind="ExternalInput")   # host-swizzled [p, kc, c]
    as2_d = nc.dram_tensor("as2", [1, OD], fp32, kind="ExternalInput")
    ad2_d = nc.dram_tensor("ad2", [1, OD], fp32, kind="ExternalInput")
    le2_d = nc.dram_tensor("le2", [EI, OD], fp32, kind="ExternalInput")
    ae2_d = nc.dram_tensor("ae2", [1, OD], fp32, kind="ExternalInput")
    b2_d = nc.dram_tensor("b2", [1, OD], fp32, kind="ExternalInput")

    # -------- intermediates / outputs --------
    h1ext = nc.dram_tensor("h1ext", [cfg.N, ROW1], bf16)
    h2ext = nc.dram_tensor("h2ext", [cfg.N, ROW2], bf16)
    z1locT = nc.dram_tensor("z1locT", [KC2, P, NBLK * P], bf16)
    z1gT = nc.dram_tensor("z1gT", [cfg.NCORES, KC2, P, NBLK * P], bf16, addr_space="Shared")
    import os as _os0
    dbgGs = (nc.dram_tensor("dbgGs", [P, M * ROW1], bf16)
             if _os0.environ.get("K_DBG") == "1" else None)
    poolpart = nc.dram_tensor("poolpart", [G, OD + 1], fp32)
    poolsum = nc.dram_tensor("poolsum", [G, OD + 1], fp32, addr_space="Shared")
    out_d = nc.dram_tensor("out", [G, OD], fp32, kind="ExternalOutput")

    with tile.TileContext(nc, num_cores=cfg.NCORES) as tc:
        import contextlib
        with contextlib.ExitStack() as stack:
            singles = stack.enter_context(tc.tile_pool(name="singles", bufs=1))

            # ---- constants / weights prep ----
            iotaF = singles.tile([P, P], bf16)
            nc.gpsimd.iota(iotaF[:], pattern=[[1, P]], base=0, channel_multiplier=0,
                           allow_small_or_imprecise_dtypes=True)
            giota = singles.tile([P, G], bf16)
            nc.gpsimd.iota(giota[:], pattern=[[1, G]], base=0, channel_multiplier=0,
                           allow_small_or_imprecise_dtypes=True)
            identb = singles.tile([P, P], bf16)
            make_identity(nc, identb[:])
            onescol = singles.tile([P, 1], bf16)
            nc.vector.memset(onescol[:], 1.0)

            attbc = singles.tile([P, C1], fp32)       # scratch broadcast
            tmpw = singles.tile([P, C1], fp32)

            # W1ext = [W1 | A_s1 | A_d1] in bf16, [NI, C1+2H]
            w1f = singles.tile([NI, C1], fp32)
            nc.sync.dma_start(out=w1f[:], in_=W1_d[:, :])
            W1ext = singles.tile([NI, C1 + 2 * H], bf16)
            nc.vector.tensor_copy(W1ext[:, 0:C1], w1f[:])

            def contract(att_dram, w_tile, kparts, ncols, nh, out_ap):
                # out[p, h] = sum_c w_tile[p, h*C + c] * att[h*C + c]
                nc.sync.dma_start(out=attbc[0:1, 0:ncols], in_=att_dram[:, :])
                nc.gpsimd.partition_broadcast(attbc[:, 0:ncols], attbc[0:1, 0:ncols])
                nc.vector.tensor_tensor(out=tmpw[0:kparts, 0:ncols], in0=w_tile,
                                        in1=attbc[0:kparts, 0:ncols], op=OP.mult)
                nc.vector.reduce_sum(
                    out=out_ap,
                    in_=tmpw[0:kparts, 0:ncols].rearrange("p (h c) -> p h c", h=nh),
                    axis=mybir.AxisListType.X)

            asd1 = singles.tile([NI, 2 * H], fp32)
            contract(as1_d, w1f[:], NI, C1, H, asd1[:, 0:H])
            contract(ad1_d, w1f[:], NI, C1, H, asd1[:, H:2 * H])
            nc.vector.tensor_copy(W1ext[:, C1:C1 + 2 * H], asd1[:])

            le1f = singles.tile([EI, C1], fp32)
            nc.sync.dma_start(out=le1f[:], in_=le1_d[:, :])
            B1f = singles.tile([EI, H], fp32)
            contract(ae1_d, le1f[:], EI, C1, H, B1f[:])
            B1 = singles.tile([EI, H], bf16)
            nc.vector.tensor_copy(B1[:], B1f[:])

            w2f = singles.tile([P, KC2 * OD], fp32)
            nc.sync.dma_start(out=w2f[:], in_=W2_d[:, :])
            W2ext = singles.tile([P, KC2, OD + 2], bf16)
            nc.vector.tensor_copy(
                W2ext[:, :, 0:OD], w2f[:].rearrange("p (k c) -> p k c", k=KC2))
            asd2 = singles.tile([P, KC2, 2], fp32)
            # layer-2 contractions: per k-chunk reduce over OD columns
            nc.sync.dma_start(out=attbc[0:1, 0:OD], in_=as2_d[:, :])
            nc.gpsimd.partition_broadcast(attbc[:, 0:OD], attbc[0:1, 0:OD])
            nc.vector.tensor_tensor(
                out=tmpw[:, 0:KC2 * OD].rearrange("p (k c) -> p k c", k=KC2),
                in0=w2f[:].rearrange("p (k c) -> p k c", k=KC2),
                in1=attbc[:, None, 0:OD].to_broadcast([P, KC2, OD]), op=OP.mult)
            nc.vector.reduce_sum(out=asd2[:, :, 0:1],
                                 in_=tmpw[:, 0:KC2 * OD].rearrange("p (k c) -> p k c", k=KC2),
                                 axis=mybir.AxisListType.X)
            nc.sync.dma_start(out=attbc[0:1, 0:OD], in_=ad2_d[:, :])
            nc.gpsimd.partition_broadcast(attbc[:, 0:OD], attbc[0:1, 0:OD])
            nc.vector.tensor_tensor(
                out=tmpw[:, 0:KC2 * OD].rearrange("p (k c) -> p k c", k=KC2),
                in0=w2f[:].rearrange("p (k c) -> p k c", k=KC2),
                in1=attbc[:, None, 0:OD].to_broadcast([P, KC2, OD]), op=OP.mult)
            nc.vector.reduce_sum(out=asd2[:, :, 1:2],
                                 in_=tmpw[:, 0:KC2 * OD].rearrange("p (k c) -> p k c", k=KC2),
                                 axis=mybir.AxisListType.X)
            nc.vector.tensor_copy(W2ext[:, :, OD:OD + 2], asd2[:])

            le2f = singles.tile([EI, OD], fp32)
            nc.sync.dma_start(out=le2f[:], in_=le2_d[:, :])
            nc.sync.dma_start(out=attbc[0:1, 0:OD], in_=ae2_d[:, :])
            nc.gpsimd.partition_broadcast(attbc[:, 0:OD], attbc[0:1, 0:OD])
            nc.vector.tensor_tensor(out=tmpw[0:EI, 0:OD], in0=le2f[:],
                                    in1=attbc[0:EI, 0:OD], op=OP.mult)
            B2f = singles.tile([EI, 1], fp32)
            nc.vector.reduce_sum(out=B2f[:], in_=tmpw[0:EI, 0:OD], axis=mybir.AxisListType.X)
            B2 = singles.tile([EI, 1], bf16)
            nc.vector.tensor_copy(B2[:], B2f[:])

            bias1bc = singles.tile([P, C1], fp32)
            nc.sync.dma_start(out=bias1bc[0:1, :], in_=b1_d[:, :])
            nc.gpsimd.partition_broadcast(bias1bc[:], bias1bc[0:1, :])
            bias2bc = singles.tile([P, OD], fp32)
            nc.sync.dma_start(out=bias2bc[0:1, :], in_=b2_d[:, :])
            nc.gpsimd.partition_broadcast(bias2bc[:], bias2bc[0:1, :])

            # persistent wrapped index arrays
            srcw = singles.tile([P, SLOTS], i16)
            nc.sync.dma_start(out=srcw[:], in_=srcw_d[:, :])
            batchw = singles.tile([P, NBLK], bf16)
            nc.sync.dma_start(out=batchw[:], in_=batchw_d[:, :])

            # ---- phase H1: h1ext / dtab1 for all N nodes ----
            XCH = 2048
            GRP = 4
            with tc.tile_pool(name="h1", bufs=2) as h1p, \
                 tc.tile_pool(name="h1ps", bufs=4, space="PSUM") as h1ps:
                for x0 in range(0, cfg.N, XCH):
                    w = min(XCH, cfg.N - x0)
                    xf = h1p.tile([P, XCH], fp32, tag="xf")
                    nc.sync.dma_start(out=xf[:, 0:w], in_=xT[:, x0:x0 + w])
                    xb = h1p.tile([P, XCH], bf16, tag="xb")
                    nc.vector.tensor_copy(xb[:, 0:w], xf[:, 0:w])
                    for t0 in range(0, w, P * GRP):
                        gw = min(P * GRP, w - t0)
                        ng = (gw + P - 1) // P
                        rowt = h1p.tile([P, GRP, ROW1], bf16, tag="rowt", bufs=3)
                        rowtf = rowt[:].bitcast(fp32)
                        if ROW1 > C1 + 2 * H:
                            nc.vector.memset(
                                rowt[:].rearrange("p g r -> p (g r)")[
                                    :, :].rearrange("p (g r) -> p g r", g=GRP)[
                                    :, :, C1 + 2 * H:ROW1], 0)
                        for gi in range(ng):
                            tw = min(P, gw - gi * P)
                            ph = h1ps.tile([P, C1 + 2 * H], fp32)
                            nc.tensor.matmul(ph[0:tw, :],
                                             xb[:, t0 + gi * P:t0 + gi * P + tw],
                                             W1ext[:], start=True, stop=True)
                            nc.scalar.copy(rowt[0:tw, gi, 0:C1], ph[0:tw, 0:C1])
                            nc.vector.tensor_copy(
                                rowtf[0:tw, gi, C1 // 2:C1 // 2 + H],
                                ph[0:tw, C1:C1 + H])
                        g0 = x0 + t0
                        full = (gw // P) * P
                        if full > 0:
                            nc.sync.dma_start(
                                out=h1ext[g0:g0 + full, :].rearrange(
                                    "(g p) r -> p g r", p=P),
                                in_=rowt[:, 0:full // P, :])
                        if gw > full:
                            nc.sync.dma_start(
                                out=h1ext[g0 + full:g0 + gw, :],
                                in_=rowt[0:gw - full, full // P, :])

            # ---- layer block processing (shared for L1 / L2) ----
            def layer_blocks(lp, lps, table, nheads, ccols, Btile, bias_bc,
                             row_elems, as_f32_off, ad_rhs, emit, gbufs=2):
                # ccols = feature cols (C1 or OD); msg row = ccols + nheads
                MR = ccols + nheads
                import os as _os
                # dma_gather crashes the device above ~1k indices per call
                GMAX = int(_os.environ.get("K_GMAX", "1024"))

                def gather(out_tile, table_ap, idx16, b, elems):
                    sl0 = b * EB // 16
                    for qi, g0 in enumerate(range(0, EB, GMAX)):
                        gn = min(GMAX, EB - g0)
                        nc.gpsimd.dma_gather(
                            out_ap=out_tile[:, g0 // P:(g0 + gn) // P, :],
                            in_ap=table_ap,
                            idxs_ap=idx16[:, sl0 + g0 // 16:sl0 + (g0 + gn) // 16],
                            num_idxs=gn, num_idxs_reg=gn, elem_size=elems,
                            queue_num=(b * 5 + qi) % _NSWQ)

                for b in range(NBLK):
                    rows = min(P, NLOC - b * P)
                    Gs = lp.tile([P, M, row_elems], bf16, tag="Gs", bufs=gbufs)
                    gather(Gs, table[:, :], srcw, b, row_elems)
                    eat = lp.tile([EI, EB], bf16, tag="eat")
                    nc.sync.dma_start(out=eat[:], in_=eaT_d[:, b * EB:(b + 1) * EB])
                    dmod = lp.tile([P, M], bf16, tag="dmod")
                    nc.sync.dma_start(out=dmod[:], in_=dmod_d[:, b * M:(b + 1) * M])

                    # per-block a_dst values of the 128 owned dst nodes, bf16 [P, nh]
                    adb = ad_rhs(b, lp, lps)

                    # selection matrices for all chunks + a_e/a_d attention logits
                    st_all = lp.tile([P, M * P], bf16, tag="st_all")
                    pae = lps.tile([P, M * nheads], fp32, tag="pae", bufs=1)
                    for c in range(M):
                        st = st_all[:, c * P:(c + 1) * P]
                        nc.vector.tensor_tensor(
                            out=st, in0=dmod[:, c:c + 1].to_broadcast([P, P]),
                            in1=iotaF[:], op=OP.is_equal)
                        psT = lps.tile([P, P], bf16, tag="psT", bufs=2)
                        nc.tensor.transpose(psT[:], st, identb[:])
                        sd = lp.tile([P, P], bf16, tag="sd", bufs=3)
                        nc.scalar.copy(sd[:], psT[:])
                        nc.tensor.matmul(pae[:, c * nheads:(c + 1) * nheads],
                                         eat[:, c * P:(c + 1) * P], Btile[:],
                                         start=True, stop=False)
                        nc.tensor.matmul(pae[:, c * nheads:(c + 1) * nheads],
                                         sd[:], adb, start=False, stop=True)
                    Gsf = Gs[:].bitcast(fp32)
                    alpha = lp.tile([P, M * nheads], fp32, tag="alpha", bufs=3)
                    nc.vector.tensor_tensor(
                        out=alpha[:].rearrange("p (m h) -> p m h", m=M),
                        in0=pae[:].rearrange("p (m h) -> p m h", m=M),
                        in1=Gsf[:, :, as_f32_off:as_f32_off + nheads], op=OP.add)
                    alr = lp.tile([P, M * nheads], fp32, tag="alr", bufs=3)
                    nc.vector.scalar_tensor_tensor(out=alr[:], in0=alpha[:], scalar=0.2,
                                                   in1=alpha[:], op0=OP.mult, op1=OP.max)
                    msgB = lp.tile([P, M * MR], bf16, tag="msgB", bufs=2)
                    # NOTE: a strided ACT output ([P, M, nheads] at stride MR) hard-
                    # crashes the device for large M; exp to a contiguous tile and
                    # spread with a DVE copy instead.
                    exb = lp.tile([P, M * nheads], bf16, tag="exb", bufs=3)
                    nc.scalar.activation(exb[:], alr[:], AF.Exp)
                    nc.vector.tensor_copy(
                        msgB[:].rearrange("p (m r) -> p m r", m=M)[:, :, ccols:MR],
                        exb[:].rearrange("p (m h) -> p m h", m=M))
                    pagg = lps.tile([P, MR], fp32, tag="pagg", bufs=2)
                    for c in range(M):
                        mrow = msgB[:].rearrange("p (m r) -> p m r", m=M)
                        nc.vector.tensor_tensor(
                            out=mrow[:, c, 0:ccols].rearrange("p (h c) -> p h c", h=nheads),
                            in0=Gs[:, c, 0:ccols].rearrange("p (h c) -> p h c", h=nheads),
                            in1=mrow[:, c, ccols:MR][:, :, None].to_broadcast(
                                [P, nheads, ccols // nheads]),
                            op=OP.mult)
                        nc.tensor.matmul(pagg[:], st_all[:, c * P:(c + 1) * P],
                                         mrow[:, c, :], start=(c == 0), stop=(c == M - 1))
                    # normalize + bias + elu
                    den = lp.tile([P, nheads], fp32, tag="den")
                    nc.vector.tensor_scalar(out=den[:], in0=pagg[:, ccols:MR],
                                            scalar1=1e-30, scalar2=None, op0=OP.max)
                    rden = lp.tile([P, nheads], fp32, tag="rden")
                    nc.vector.reciprocal(rden[:], den[:])
                    zp = lp.tile([P, ccols], fp32, tag="zp")
                    hc = ccols // nheads
                    for h in range(nheads):
                        nc.scalar.activation(zp[:, h * hc:(h + 1) * hc],
                                             pagg[:, h * hc:(h + 1) * hc], AF.Copy,
                                             scale=rden[:, h:h + 1])
                    nc.vector.tensor_tensor(out=zp[:], in0=zp[:], in1=bias_bc, op=OP.add)
                    mneg = lp.tile([P, ccols], fp32, tag="mneg")
                    nc.vector.scalar_tensor_tensor(out=mneg[:], in0=zp[:], scalar=0.0,
                                                   in1=zp[:], op0=OP.mult, op1=OP.min)
                    em = lp.tile([P, ccols], fp32, tag="em")
                    nc.scalar.activation(em[:], mneg[:], AF.Exp)
                    zf = lp.tile([P, ccols], fp32, tag="zf")
                    nc.vector.scalar_tensor_tensor(out=zf[:], in0=em[:], scalar=-1.0,
                                                   in1=zp[:], op0=OP.add, op1=OP.add)
                    nc.vector.tensor_tensor(out=zf[:], in0=zf[:], in1=mneg[:],
                                            op=OP.subtract)
                    emit(b, rows, zf, lp, lps)

            # ---- L1 ----
            def emit1(b, rows, zf, lp, lps):
                z1b = lp.tile([P, C1], bf16, tag="z1b")
                nc.scalar.copy(z1b[:], zf[:])
                for kc in range(KC2):
                    pt = lps.tile([P, P], bf16, tag="pt")
                    nc.tensor.transpose(pt[:], z1b[:, kc * P:(kc + 1) * P], identb[:])
                    zt = lp.tile([P, P], bf16, tag="zt", bufs=3)
                    nc.scalar.copy(zt[:], pt[:])
                    nc.sync.dma_start(out=z1locT[kc, :, b * P:(b + 1) * P],
                                      in_=zt[:, :])

            def ad_rhs1(b, lp, lps):
                xtl = lp.tile([P, P], fp32, tag="xtl")
                nc.sync.dma_start(out=xtl[:], in_=xTloc_d[:, b * P:(b + 1) * P])
                xtlb = lp.tile([P, P], bf16, tag="xtlb")
                nc.vector.tensor_copy(xtlb[:], xtl[:])
                pad = lps.tile([P, H], fp32, tag="pad", bufs=1)
                nc.tensor.matmul(pad[:], xtlb[:], W1ext[:, C1 + H:C1 + 2 * H],
                                 start=True, stop=True)
                adb = lp.tile([P, H], bf16, tag="adb")
                nc.scalar.copy(adb[:], pad[:])
                return adb[:]

            with tc.tile_pool(name="l1", bufs=2) as lp, \
                 tc.tile_pool(name="l1ps", bufs=2, space="PSUM") as lps:
                layer_blocks(lp, lps, h1ext, H, C1, B1, bias1bc[:],
                             ROW1, C1 // 2, ad_rhs1, emit1, gbufs=3)

            # ---- AllGather z1 ----
            nc.gpsimd.collective_compute(
                "AllGather", mybir.AluOpType.bypass, replica_groups=groups,
                ins=[z1locT.ap()], outs=[z1gT.ap()])

            # ---- phase H2: h2ext rows for all N nodes ----
            with tc.tile_pool(name="h2", bufs=3) as h2p, \
                 tc.tile_pool(name="h2ps", bufs=4, space="PSUM") as h2ps:
                for k in range(cfg.NCORES):
                    # one big slab load per (remote core, k-chunk): [P, NBLK*P]
                    slabs = []
                    for kc in range(KC2):
                        sl = h2p.tile([P, NBLK * P], bf16, tag=f"slab{kc}", bufs=2)
                        nc.sync.dma_start(out=sl[:], in_=z1gT[k, kc, :, :])
                        slabs.append(sl)
                    G2 = 4
                    for b0 in range(0, NBLK, G2):
                        nb = min(G2, NBLK - b0)
                        row2 = h2p.tile([P, G2, ROW2], bf16, tag="row2", bufs=3)
                        row2f = row2[:].bitcast(fp32)
                        nc.vector.memset(row2[:].rearrange("p g r -> p (g r)"), 0)
                        for gi in range(nb):
                            b = b0 + gi
                            w = min(P, NLOC - b * P)
                            ph = h2ps.tile([P, OD + 2], fp32)
                            for kc in range(KC2):
                                nc.tensor.matmul(ph[0:w, :],
                                                 slabs[kc][:, b * P:(b + 1) * P][:, 0:w],
                                                 W2ext[:, kc, :],
                                                 start=(kc == 0), stop=(kc == KC2 - 1))
                            nc.scalar.copy(row2[0:w, gi, 0:OD], ph[0:w, 0:OD])
                            nc.vector.tensor_copy(row2f[0:w, gi, OD // 2:OD // 2 + 2],
                                                  ph[0:w, OD:OD + 2])
                        g0 = k * NLOC + b0 * P
                        gw = min(G2 * P, NLOC - b0 * P)
                        full = (gw // P) * P
                        if full > 0:
                            nc.sync.dma_start(
                                out=h2ext[g0:g0 + full, :].rearrange(
                                    "(g p) r -> p g r", p=P),
                                in_=row2[:, 0:full // P, :])
                        if gw > full:
                            nc.sync.dma_start(
                                out=h2ext[g0 + full:g0 + gw, :],
                                in_=row2[0:gw - full, full // P, :])

            # ---- L2 + pool ----
            ppool = None

            def emit2(b, rows, zf, lp, lps):
                nonlocal ppool
                pb = lp.tile([P, OD + 1], bf16, tag="pb")
                nc.scalar.copy(pb[:, 0:OD], zf[:])
                nc.vector.tensor_copy(pb[:, OD:OD + 1], onescol[:])
                sbg = lp.tile([P, G], bf16, tag="sbg")
                nc.vector.tensor_tensor(out=sbg[:],
                                        in0=batchw[:, b:b + 1].to_broadcast([P, G]),
                                        in1=giota[:], op=OP.is_equal)
                nc.tensor.matmul(ppool[:], sbg[:], pb[:],
                                 start=(b == 0), stop=(b == NBLK - 1))

            def ad_rhs2(b, lp, lps):
                pad = lps.tile([P, 1], fp32, tag="pad", bufs=1)
                for kc in range(KC2):
                    zl = lp.tile([P, P], bf16, tag="zl")
                    nc.sync.dma_start(out=zl[:], in_=z1locT[kc, :, b * P:(b + 1) * P])
                    nc.tensor.matmul(pad[:], zl[:], W2ext[:, kc, OD + 1:OD + 2],
                                     start=(kc == 0), stop=(kc == KC2 - 1))
                adb = lp.tile([P, 1], bf16, tag="adb")
                nc.scalar.copy(adb[:], pad[:])
                return adb[:]

            with tc.tile_pool(name="l2", bufs=2) as lp2, \
                 tc.tile_pool(name="l2ps", bufs=2, space="PSUM") as lps2:
                ppool = lps2.tile([G, OD + 1], fp32, tag="ppool")
                layer_blocks(lp2, lps2, h2ext, 1, OD, B2, bias2bc[:],
                             ROW2, OD // 2, ad_rhs2, emit2, gbufs=3)
                pls = lp2.tile([G, OD + 1], fp32, tag="pls")
                nc.vector.tensor_copy(pls[:], ppool[:])
                nc.sync.dma_start(out=poolpart[:, :], in_=pls[:])

            # ---- AllReduce pool, final mean ----
            nc.gpsimd.collective_compute(
                "AllReduce", mybir.AluOpType.add, replica_groups=groups,
                ins=[poolpart.ap()], outs=[poolsum.ap()])
            fin = singles.tile([G, OD + 1], fp32)
            nc.sync.dma_start(out=fin[:], in_=poolsum[:, :])
            cnt = singles.tile([G, 1], fp32)
            nc.vector.tensor_scalar(out=cnt[:], in0=fin[:, OD:OD + 1], scalar1=1.0,
                                    scalar2=None, op0=OP.max)
            rcnt = singles.tile([G, 1], fp32)
            nc.vector.reciprocal(rcnt[:], cnt[:])
            outs = singles.tile([G, OD], fp32)
            nc.scalar.activation(outs[:], fin[:, 0:OD], AF.Copy, scale=rcnt[:])
            nc.sync.dma_start(out=out_d[:, :], in_=outs[:])

    nc.compile()
    return nc


# ---------------- in_maps assembly ----------------------------------------
def _in_maps(cfg, per_core, inputs):
    xT = np.ascontiguousarray(np.asarray(inputs["x"], np.float32).T)
    KC2 = cfg.KC2
    W2 = np.asarray(inputs["W2"], np.float32).reshape(KC2, P, cfg.OUT_DIM)
    W2s = np.ascontiguousarray(W2.transpose(1, 0, 2).reshape(P, KC2 * cfg.OUT_DIM))
    common = dict(
        xT=xT,
        W1=np.asarray(inputs["W1"], np.float32),
        as1=np.asarray(inputs["att_src1"], np.float32).reshape(1, -1),
        ad1=np.asarray(inputs["att_dst1"], np.float32).reshape(1, -1),
        le1=np.asarray(inputs["lin_edge1"], np.float32),
        ae1=np.asarray(inputs["att_edge1"], np.float32).reshape(1, -1),
        b1=np.asarray(inputs["bias1"], np.float32).reshape(1, -1),
        W2=W2s,
        as2=np.asarray(inputs["att_src2"], np.float32).reshape(1, -1),
        ad2=np.asarray(inputs["att_dst2"], np.float32).reshape(1, -1),
        le2=np.asarray(inputs["lin_edge2"], np.float32),
        ae2=np.asarray(inputs["att_edge2"], np.float32).reshape(1, -1),
        b2=np.asarray(inputs["bias2"], np.float32).reshape(1, -1),
    )
    maps = []
    NBP = cfg.NBLK * P
    for k in range(cfg.NCORES):
        pc = per_core[k]
        m = dict(common)
        xTloc = np.zeros((cfg.NODE_IN, NBP), np.float32)
        xTloc[:, :cfg.NLOC] = xT[:, k * cfg.NLOC:(k + 1) * cfg.NLOC]
        m.update(srcw=pc["srcw"], dstw=pc["dstw"], xTloc=xTloc, dmod=pc["dmod"],
                 eaT=pc["eaT"], batchw=pc["batchw"])
        maps.append(m)
    return maps


_CACHE = {}


def run(cfg, inputs, trace=False):
    from concourse.bass_utils import run_bass_kernel_spmd
    per_core, M, SLOTS, EPAD = _prep(cfg, inputs["edge_index"], inputs["edge_attr"],
                                     inputs["batch"])
    key = (cfg.N, cfg.E, M)
    if key not in _CACHE:
        _CACHE[key] = _build(cfg, M)
    nc = _CACHE[key]
    maps = _in_maps(cfg, per_core, inputs)
    res = run_bass_kernel_spmd(nc, maps, core_ids=list(range(cfg.NCORES)),
                               trace=trace)
    return res


def kernel(**inputs):
    cfg = Cfg()
    res = run(cfg, inputs)
    return np.asarray(res.results[0]["out"], np.float32)


# revision 42
# speedup vs baseline: 1.0029x; 1.0029x over previous
# Trainium2 Bass kernel for a 2-layer edge-featured GAT (PyG GATConv style)
# with global mean pool, distributed over 8 NeuronCores.
#
# Sharding: edges are partitioned by destination node range (dst-shard); each
# core owns N/8 consecutive dst nodes and all edges pointing into them, so the
# segment softmax / aggregation is core-local.  Node features are computed
# replicated.  Layer-1 output is exchanged with an AllGather; the graph pool
# uses an AllReduce of per-core partial (sum, count) tables.
#
# Per-core algorithm (per layer):
#   h_ext = x @ [W | A_src | A_dst]      (A_* = W contracted with att vectors)
#   gather h_ext rows by edge src (dma_gather, spread over 4 SWDGE queues);
#   a_dst reaches edges via a per-block matmul against the transposed
#   selection matrix (no dst-side gather at all)
#   alpha = leaky_relu(a_src[src] + a_dst[dst] + ea @ B, 0.2); ex = exp(alpha)
#   (softmax max-subtraction is skipped: it cancels exactly in the ratio and
#    alpha is bounded by the data scale, far from fp32 overflow)
#   per 128-dst block: agg = sum_chunks S_chunk^T @ [ex*h | ex]   (PE matmuls,
#    S = one-hot dst selection built by a DVE compare against an iota)
#   out_row = elu(agg[:, :C] / agg[:, C:] + bias)

import numpy as np
import ml_dtypes

BF16 = ml_dtypes.bfloat16

# ---------------- problem constants (self-contained; must match grader) -----
N, E, NODE_IN, EDGE_IN, HIDDEN, HEADS, OUT_DIM, NUM_GRAPHS = (
    20000, 640000, 128, 16, 64, 4, 64, 64)
NCORES = 8
P = 128


class Cfg:
    def __init__(self, n=N, e=E, node_in=NODE_IN, edge_in=EDGE_IN, hidden=HIDDEN,
                 heads=HEADS, out_dim=OUT_DIM, num_graphs=NUM_GRAPHS, ncores=NCORES):
        self.N, self.E, self.NODE_IN, self.EDGE_IN = n, e, node_in, edge_in
        self.HIDDEN, self.HEADS, self.OUT_DIM, self.NUM_GRAPHS = hidden, heads, out_dim, num_graphs
        self.NCORES = ncores
        assert n % ncores == 0
        self.NLOC = n // ncores
        self.NBLK = (self.NLOC + P - 1) // P
        self.C1 = heads * hidden                      # 256
        self.ROW1 = self.C1 + 8                       # h1 bf16 + a_s (4 f32 = 8 bf16 slots)
        # round ROW1 (bf16 elements) so row bytes % 256 == 0
        self.ROW1 = ((self.ROW1 * 2 + 255) // 256) * 256 // 2       # 384
        self.ROW2 = ((self.OUT_DIM + 4 + 127) // 128) * 128         # bf16 elems, 256B -> 128
        self.KC2 = (self.C1 + P - 1) // P             # k-chunks for layer-2 matmul


# ---------------- host-side sharding prep (integer index work only) ---------
def _prep(cfg, edge_index, edge_attr, batch):
    n, e = cfg.N, cfg.E
    src = np.concatenate([np.asarray(edge_index[0]), np.arange(n)]).astype(np.int64)
    dst = np.concatenate([np.asarray(edge_index[1]), np.arange(n)]).astype(np.int64)
    ea = np.asarray(edge_attr, dtype=np.float32)
    mean_ea = ea.mean(axis=0, keepdims=True)
    ea_all = np.concatenate([ea, np.broadcast_to(mean_ea, (n, cfg.EDGE_IN))], axis=0)

    core = dst // cfg.NLOC
    dloc = dst - core * cfg.NLOC
    blk = dloc // P

    # max chunks per (core, block)
    M = 1
    buckets = {}
    for k in range(cfg.NCORES):
        mk = core == k
        for b in range(cfg.NBLK):
            sel = np.nonzero(mk & (blk == b))[0]
            buckets[(k, b)] = sel
            M = max(M, (len(sel) + P - 1) // P)
    EB = M * P
    EPAD = cfg.NBLK * EB
    SLOTS = EPAD // 16

    per_core = []
    for k in range(cfg.NCORES):
        srcp = np.zeros(EPAD, np.int16)
        dstp = np.zeros(EPAD, np.int16)
        dmod = np.full(EPAD, -1.0, np.float32)
        eaT = np.zeros((cfg.EDGE_IN, EPAD), np.float32)
        for b in range(cfg.NBLK):
            sel = buckets[(k, b)]
            o = b * EB
            nn = len(sel)
            srcp[o:o + nn] = src[sel]
            dstp[o:o + nn] = dst[sel]
            dmod[o:o + nn] = (dloc[sel] - b * P).astype(np.float32)
            eaT[:, o:o + nn] = ea_all[sel].T
        # wrapped idx layout: idx j -> partition j%16, slot j//16; replicated x8
        srcw = np.tile(srcp.reshape(-1, 16).T, (8, 1)).copy()     # [128, SLOTS]
        dstw = np.tile(dstp.reshape(-1, 16).T, (8, 1)).copy()
        # chunk-major layout: [p, b*M+c] = edge (b, c, p) — for indirect DMA
        src32 = srcp.reshape(cfg.NBLK * M, P).T.astype(np.int32).copy()
        dst32 = dstp.reshape(cfg.NBLK * M, P).T.astype(np.int32).copy()
        dmod_hm = dmod.reshape(cfg.NBLK * M, P).T.astype(BF16).copy()   # [128, NBLK*M]
        # batch ids of this core's nodes, [128, NBLK], pad -1
        bw = np.full((P, cfg.NBLK), -1.0, np.float32)
        bvals = np.asarray(batch[k * cfg.NLOC:(k + 1) * cfg.NLOC], dtype=np.float32)
        for b in range(cfg.NBLK):
            seg = bvals[b * P:(b + 1) * P]
            bw[:len(seg), b] = seg
        per_core.append(dict(
            srcw=srcw, dstw=dstw, src32=src32, dst32=dst32, dmod=dmod_hm,
            eaT=eaT.astype(BF16), batchw=bw.astype(BF16)))
    return per_core, M, SLOTS, EPAD


# ---------------- device program ------------------------------------------
def _build(cfg, M):
    import concourse.bass as bass
    import concourse.mybir as mybir
    import concourse.tile as tile
    from concourse import bacc
    from concourse.masks import make_identity

    fp32 = mybir.dt.float32
    bf16 = mybir.dt.bfloat16
    i16 = mybir.dt.int16
    AF = mybir.ActivationFunctionType
    OP = mybir.AluOpType

    H, C, OD = cfg.HEADS, cfg.HIDDEN, cfg.OUT_DIM
    C1 = cfg.C1
    NBLK, NLOC, NI, EI = cfg.NBLK, cfg.NLOC, cfg.NODE_IN, cfg.EDGE_IN
    EB = M * P
    EPAD = NBLK * EB
    SLOTS = EPAD // 16
    NT = (cfg.N + P - 1) // P          # global node tiles
    G = cfg.NUM_GRAPHS
    ROW1, ROW2, KC2 = cfg.ROW1, cfg.ROW2, cfg.KC2

    import os as _osq
    _NSWQ = int(_osq.environ.get("K_QN", "4"))
    nc = bacc.Bacc("TRN2", target_bir_lowering=False, debug=False,
                   enable_asserts=False, num_devices=cfg.NCORES,
                   num_swdge_queues=_NSWQ)
    groups = [list(range(cfg.NCORES))]

    # -------- external inputs --------
    xT = nc.dram_tensor("xT", [NI, cfg.N], fp32, kind="ExternalInput")
    srcw_d = nc.dram_tensor("srcw", [P, SLOTS], i16, kind="ExternalInput")
    xTloc_d = nc.dram_tensor("xTloc", [NI, NBLK * P], fp32, kind="ExternalInput")
    dmod_d = nc.dram_tensor("dmod", [P, NBLK * M], bf16, kind="ExternalInput")
    eaT_d = nc.dram_tensor("eaT", [EI, EPAD], bf16, kind="ExternalInput")
    batchw_d = nc.dram_tensor("batchw", [P, NBLK], bf16, kind="ExternalInput")
    W1_d = nc.dram_tensor("W1", [NI, C1], fp32, kind="ExternalInput")
    as1_d = nc.dram_tensor("as1", [1, C1], fp32, kind="ExternalInput")
    ad1_d = nc.dram_tensor("ad1", [1, C1], fp32, kind="ExternalInput")
    le1_d = nc.dram_tensor("le1", [EI, C1], fp32, kind="ExternalInput")
    ae1_d = nc.dram_tensor("ae1", [1, C1], fp32, kind="ExternalInput")
    b1_d = nc.dram_tensor("b1", [1, C1], fp32, kind="ExternalInput")
    W2_d = nc.dram_tensor("W2", [P, KC2 * OD], fp32, kind="ExternalInput")   # host-swizzled [p, kc, c]
    as2_d = nc.dram_tensor("as2", [1, OD], fp32, kind="ExternalInput")
    ad2_d = nc.dram_tensor("ad2", [1, OD], fp32, kind="ExternalInput")
    le2_d = nc.dram_tensor("le2", [EI, OD], fp32, kind="ExternalInput")
    ae2_d = nc.dram_tensor("ae2", [1, OD], fp32, kind="ExternalInput")
    b2_d = nc.dram_tensor("b2", [1, OD], fp32, kind="ExternalInput")

    # -------- intermediates / outputs --------
    h1ext = nc.dram_tensor("h1ext", [cfg.N, ROW1], bf16)
    h2ext = nc.dram_tensor("h2ext", [cfg.N, ROW2], bf16)
    z1locT = nc.dram_tensor("z1locT", [KC2, P, NBLK * P], bf16)
    z1gT = nc.dram_tensor("z1gT", [cfg.NCORES, KC2, P, NBLK * P], bf16, addr_space="Shared")
    import os as _os0
    dbgGs = (nc.dram_tensor("dbgGs", [P, M * ROW1], bf16)
             if _os0.environ.get("K_DBG") == "1" else None)
    poolpart = nc.dram_tensor("poolpart", [G, OD + 1], fp32)
    poolsum = nc.dram_tensor("poolsum", [G, OD + 1], fp32, addr_space="Shared")
    out_d = nc.dram_tensor("out", [G, OD], fp32, kind="ExternalOutput")

    with tile.TileContext(nc, num_cores=cfg.NCORES) as tc:
        import contextlib
        with contextlib.ExitStack() as stack:
            singles = stack.enter_context(tc.tile_pool(name="singles", bufs=1))

            # ---- constants / weights prep ----
            iotaF = singles.tile([P, P], bf16)
            nc.gpsimd.iota(iotaF[:], pattern=[[1, P]], base=0, channel_multiplier=0,
                           allow_small_or_imprecise_dtypes=True)
            giota = singles.tile([P, G], bf16)
            nc.gpsimd.iota(giota[:], pattern=[[1, G]], base=0, channel_multiplier=0,
                           allow_small_or_imprecise_dtypes=True)
            identb = singles.tile([P, P], bf16)
            make_identity(nc, identb[:])
            onescol = singles.tile([P, 1], bf16)
            nc.vector.memset(onescol[:], 1.0)

            attbc = singles.tile([P, C1], fp32)       # scratch broadcast
            tmpw = singles.tile([P, C1], fp32)

            # W1ext = [W1 | A_s1 | A_d1] in bf16, [NI, C1+2H]
            w1f = singles.tile([NI, C1], fp32)
            nc.sync.dma_start(out=w1f[:], in_=W1_d[:, :])
            W1ext = singles.tile([NI, C1 + 2 * H], bf16)
            nc.vector.tensor_copy(W1ext[:, 0:C1], w1f[:])

            def contract(att_dram, w_tile, kparts, ncols, nh, out_ap):
                # out[p, h] = sum_c w_tile[p, h*C + c] * att[h*C + c]
                nc.sync.dma_start(out=attbc[0:1, 0:ncols], in_=att_dram[:, :])
                nc.gpsimd.partition_broadcast(attbc[:, 0:ncols], attbc[0:1, 0:ncols])
                nc.vector.tensor_tensor(out=tmpw[0:kparts, 0:ncols], in0=w_tile,
                                        in1=attbc[0:kparts, 0:ncols], op=OP.mult)
                nc.vector.reduce_sum(
                    out=out_ap,
                    in_=tmpw[0:kparts, 0:ncols].rearrange("p (h c) -> p h c", h=nh),
                    axis=mybir.AxisListType.X)

            asd1 = singles.tile([NI, 2 * H], fp32)
            contract(as1_d, w1f[:], NI, C1, H, asd1[:, 0:H])
            contract(ad1_d, w1f[:], NI, C1, H, asd1[:, H:2 * H])
            nc.vector.tensor_copy(W1ext[:, C1:C1 + 2 * H], asd1[:])

            le1f = singles.tile([EI, C1], fp32)
            nc.sync.dma_start(out=le1f[:], in_=le1_d[:, :])
            B1f = singles.tile([EI, H], fp32)
            contract(ae1_d, le1f[:], EI, C1, H, B1f[:])
            B1 = singles.tile([EI, H], bf16)
            nc.vector.tensor_copy(B1[:], B1f[:])

            w2f = singles.tile([P, KC2 * OD], fp32)
            nc.sync.dma_start(out=w2f[:], in_=W2_d[:, :])
            W2ext = singles.tile([P, KC2, OD + 2], bf16)
            nc.vector.tensor_copy(
                W2ext[:, :, 0:OD], w2f[:].rearrange("p (k c) -> p k c", k=KC2))
            asd2 = singles.tile([P, KC2, 2], fp32)
            # layer-2 contractions: per k-chunk reduce over OD columns
            nc.sync.dma_start(out=attbc[0:1, 0:OD], in_=as2_d[:, :])
            nc.gpsimd.partition_broadcast(attbc[:, 0:OD], attbc[0:1, 0:OD])
            nc.vector.tensor_tensor(
                out=tmpw[:, 0:KC2 * OD].rearrange("p (k c) -> p k c", k=KC2),
                in0=w2f[:].rearrange("p (k c) -> p k c", k=KC2),
                in1=attbc[:, None, 0:OD].to_broadcast([P, KC2, OD]), op=OP.mult)
            nc.vector.reduce_sum(out=asd2[:, :, 0:1],
                                 in_=tmpw[:, 0:KC2 * OD].rearrange("p (k c) -> p k c", k=KC2),
                                 axis=mybir.AxisListType.X)
            nc.sync.dma_start(out=attbc[0:1, 0:OD], in_=ad2_d[:, :])
            nc.gpsimd.partition_broadcast(attbc[:, 0:OD], attbc[0:1, 0:OD])
            nc.vector.tensor_tensor(
                out=tmpw[:, 0:KC2 * OD].rearrange("p (k c) -> p k c", k=KC2),
                in0=w2f[:].rearrange("p (k c) -> p k c", k=KC2),
                in1=attbc[:, None, 0:OD].to_broadcast([P, KC2, OD]), op=OP.mult)
            nc.vector.reduce_sum(out=asd2[:, :, 1:2],
                                 in_=tmpw[:, 0:KC2 * OD].rearrange("p (k c) -> p k c", k=KC2),
                                 axis=mybir.AxisListType.X)
            nc.vector.tensor_copy(W2ext[:, :, OD:OD + 2], asd2[:])

            le2f = singles.tile([EI, OD], fp32)
            nc.sync.dma_start(out=le2f[:], in_=le2_d[:, :])
            nc.sync.dma_start(out=attbc[0:1, 0:OD], in_=ae2_d[:, :])
            nc.gpsimd.partition_broadcast(attbc[:, 0:OD], attbc[0:1, 0:OD])
            nc.vector.tensor_tensor(out=tmpw[0:EI, 0:OD], in0=le2f[:],
                                    in1=attbc[0:EI, 0:OD], op=OP.mult)
            B2f = singles.tile([EI, 1], fp32)
            nc.vector.reduce_sum(out=B2f[:], in_=tmpw[0:EI, 0:OD], axis=mybir.AxisListType.X)
            B2 = singles.tile([EI, 1], bf16)
            nc.vector.tensor_copy(B2[:], B2f[:])

            bias1bc = singles.tile([P, C1], fp32)
            nc.sync.dma_start(out=bias1bc[0:1, :], in_=b1_d[:, :])
            nc.gpsimd.partition_broadcast(bias1bc[:], bias1bc[0:1, :])
            bias2bc = singles.tile([P, OD], fp32)
            nc.sync.dma_start(out=bias2bc[0:1, :], in_=b2_d[:, :])
            nc.gpsimd.partition_broadcast(bias2bc[:], bias2bc[0:1, :])

            # persistent wrapped index arrays
            srcw = singles.tile([P, SLOTS], i16)
            nc.sync.dma_start(out=srcw[:], in_=srcw_d[:, :])
            batchw = singles.tile([P, NBLK], bf16)
            nc.sync.dma_start(out=batchw[:], in_=batchw_d[:, :])

            # ---- phase H1: h1ext / dtab1 for all N nodes ----
            XCH = 2048
            GRP = 4
            with tc.tile_pool(name="h1", bufs=2) as h1p, \
                 tc.tile_pool(name="h1ps", bufs=4, space="PSUM") as h1ps:
                for x0 in range(0, cfg.N, XCH):
                    w = min(XCH, cfg.N - x0)
                    xf = h1p.tile([P, XCH], fp32, tag="xf")
                    nc.sync.dma_start(out=xf[:, 0:w], in_=xT[:, x0:x0 + w])
                    xb = h1p.tile([P, XCH], bf16, tag="xb")
                    nc.vector.tensor_copy(xb[:, 0:w], xf[:, 0:w])
                    for t0 in range(0, w, P * GRP):
                        gw = min(P * GRP, w - t0)
                        ng = (gw + P - 1) // P
                        rowt = h1p.tile([P, GRP, ROW1], bf16, tag="rowt", bufs=3)
                        rowtf = rowt[:].bitcast(fp32)
                        if ROW1 > C1 + 2 * H:
                            nc.vector.memset(
                                rowt[:].rearrange("p g r -> p (g r)")[
                                    :, :].rearrange("p (g r) -> p g r", g=GRP)[
                                    :, :, C1 + 2 * H:ROW1], 0)
                        for gi in range(ng):
                            tw = min(P, gw - gi * P)
                            ph = h1ps.tile([P, C1 + 2 * H], fp32)
                            nc.tensor.matmul(ph[0:tw, :],
                                             xb[:, t0 + gi * P:t0 + gi * P + tw],
                                             W1ext[:], start=True, stop=True)
                            nc.scalar.copy(rowt[0:tw, gi, 0:C1], ph[0:tw, 0:C1])
                            nc.vector.tensor_copy(
                                rowtf[0:tw, gi, C1 // 2:C1 // 2 + H],
                                ph[0:tw, C1:C1 + H])
                        g0 = x0 + t0
                        full = (gw // P) * P
                        if full > 0:
                            nc.sync.dma_start(
                                out=h1ext[g0:g0 + full, :].rearrange(
                                    "(g p) r -> p g r", p=P),
                                in_=rowt[:, 0:full // P, :])
                        if gw > full:
                            nc.sync.dma_start(
                                out=h1ext[g0 + full:g0 + gw, :],
                                in_=rowt[0:gw - full, full // P, :])

            # ---- layer block processing (shared for L1 / L2) ----
            def layer_blocks(lp, lps, table, nheads, ccols, Btile, bias_bc,
                             row_elems, as_f32_off, ad_rhs, emit, gbufs=2):
                # ccols = feature cols (C1 or OD); msg row = ccols + nheads
                MR = ccols + nheads
                import os as _os
                # dma_gather crashes the device above ~1k indices per call
                GMAX = int(_os.environ.get("K_GMAX", "1024"))

                def gather(out_tile, table_ap, idx16, b, elems):
                    sl0 = b * EB // 16
                    for qi, g0 in enumerate(range(0, EB, GMAX)):
                        gn = min(GMAX, EB - g0)
                        nc.gpsimd.dma_gather(
                            out_ap=out_tile[:, g0 // P:(g0 + gn) // P, :],
                            in_ap=table_ap,
                            idxs_ap=idx16[:, sl0 + g0 // 16:sl0 + (g0 + gn) // 16],
                            num_idxs=gn, num_idxs_reg=gn, elem_size=elems,
                            queue_num=(b * 5 + qi) % _NSWQ)

                for b in range(NBLK):
                    rows = min(P, NLOC - b * P)
                    Gs = lp.tile([P, M, row_elems], bf16, tag="Gs", bufs=gbufs)
                    gather(Gs, table[:, :], srcw, b, row_elems)
                    eat = lp.tile([EI, EB], bf16, tag="eat", bufs=3)
                    nc.sync.dma_start(out=eat[:], in_=eaT_d[:, b * EB:(b + 1) * EB])
                    dmod = lp.tile([P, M], bf16, tag="dmod", bufs=3)
                    nc.sync.dma_start(out=dmod[:], in_=dmod_d[:, b * M:(b + 1) * M])

                    # per-block a_dst values of the 128 owned dst nodes, bf16 [P, nh]
                    adb = ad_rhs(b, lp, lps)

                    # selection matrices for all chunks + a_e/a_d attention logits
                    st_all = lp.tile([P, M * P], bf16, tag="st_all")
                    pae = lps.tile([P, M * nheads], fp32, tag="pae", bufs=1)
                    for c in range(M):
                        st = st_all[:, c * P:(c + 1) * P]
                        nc.vector.tensor_tensor(
                            out=st, in0=dmod[:, c:c + 1].to_broadcast([P, P]),
                            in1=iotaF[:], op=OP.is_equal)
                        psT = lps.tile([P, P], bf16, tag="psT", bufs=2)
                        nc.tensor.transpose(psT[:], st, identb[:])
                        sd = lp.tile([P, P], bf16, tag="sd", bufs=4)
                        nc.scalar.copy(sd[:], psT[:])
                        nc.tensor.matmul(pae[:, c * nheads:(c + 1) * nheads],
                                         eat[:, c * P:(c + 1) * P], Btile[:],
                                         start=True, stop=False)
                        nc.tensor.matmul(pae[:, c * nheads:(c + 1) * nheads],
                                         sd[:], adb, start=False, stop=True)
                    Gsf = Gs[:].bitcast(fp32)
                    alpha = lp.tile([P, M * nheads], fp32, tag="alpha", bufs=3)
                    nc.vector.tensor_tensor(
                        out=alpha[:].rearrange("p (m h) -> p m h", m=M),
                        in0=pae[:].rearrange("p (m h) -> p m h", m=M),
                        in1=Gsf[:, :, as_f32_off:as_f32_off + nheads], op=OP.add)
                    alr = lp.tile([P, M * nheads], fp32, tag="alr", bufs=3)
                    nc.vector.scalar_tensor_tensor(out=alr[:], in0=alpha[:], scalar=0.2,
                                                   in1=alpha[:], op0=OP.mult, op1=OP.max)
                    msgB = lp.tile([P, M * MR], bf16, tag="msgB", bufs=2)
                    # NOTE: a strided ACT output ([P, M, nheads] at stride MR) hard-
                    # crashes the device for large M; exp to a contiguous tile and
                    # spread with a DVE copy instead.
                    exb = lp.tile([P, M * nheads], bf16, tag="exb", bufs=3)
                    nc.scalar.activation(exb[:], alr[:], AF.Exp)
                    nc.vector.tensor_copy(
                        msgB[:].rearrange("p (m r) -> p m r", m=M)[:, :, ccols:MR],
                        exb[:].rearrange("p (m h) -> p m h", m=M))
                    pagg = lps.tile([P, MR], fp32, tag="pagg", bufs=2)
                    for c in range(M):
                        mrow = msgB[:].rearrange("p (m r) -> p m r", m=M)
                        nc.vector.tensor_tensor(
                            out=mrow[:, c, 0:ccols].rearrange("p (h c) -> p h c", h=nheads),
                            in0=Gs[:, c, 0:ccols].rearrange("p (h c) -> p h c", h=nheads),
                            in1=mrow[:, c, ccols:MR][:, :, None].to_broadcast(
                                [P, nheads, ccols // nheads]),
                            op=OP.mult)
                        nc.tensor.matmul(pagg[:], st_all[:, c * P:(c + 1) * P],
                                         mrow[:, c, :], start=(c == 0), stop=(c == M - 1))
                    # normalize + bias + elu
                    den = lp.tile([P, nheads], fp32, tag="den")
                    nc.vector.tensor_scalar(out=den[:], in0=pagg[:, ccols:MR],
                                            scalar1=1e-30, scalar2=None, op0=OP.max)
                    rden = lp.tile([P, nheads], fp32, tag="rden")
                    nc.vector.reciprocal(rden[:], den[:])
                    zp = lp.tile([P, ccols], fp32, tag="zp")
                    hc = ccols // nheads
                    for h in range(nheads):
                        nc.scalar.activation(zp[:, h * hc:(h + 1) * hc],
                                             pagg[:, h * hc:(h + 1) * hc], AF.Copy,
                                             scale=rden[:, h:h + 1])
                    nc.vector.tensor_tensor(out=zp[:], in0=zp[:], in1=bias_bc, op=OP.add)
                    mneg = lp.tile([P, ccols], fp32, tag="mneg")
                    nc.vector.scalar_tensor_tensor(out=mneg[:], in0=zp[:], scalar=0.0,
                                                   in1=zp[:], op0=OP.mult, op1=OP.min)
                    em = lp.tile([P, ccols], fp32, tag="em")
                    nc.scalar.activation(em[:], mneg[:], AF.Exp)
                    zf = lp.tile([P, ccols], fp32, tag="zf")
                    nc.vector.scalar_tensor_tensor(out=zf[:], in0=em[:], scalar=-1.0,
                                                   in1=zp[:], op0=OP.add, op1=OP.add)
                    nc.vector.tensor_tensor(out=zf[:], in0=zf[:], in1=mneg[:],
                                            op=OP.subtract)
                    emit(b, rows, zf, lp, lps)

            # ---- L1 ----
            def emit1(b, rows, zf, lp, lps):
                z1b = lp.tile([P, C1], bf16, tag="z1b")
                nc.scalar.copy(z1b[:], zf[:])
                for kc in range(KC2):
                    pt = lps.tile([P, P], bf16, tag="pt")
                    nc.tensor.transpose(pt[:], z1b[:, kc * P:(kc + 1) * P], identb[:])
                    zt = lp.tile([P, P], bf16, tag="zt", bufs=3)
                    nc.scalar.copy(zt[:], pt[:])
                    nc.sync.dma_start(out=z1locT[kc, :, b * P:(b + 1) * P],
                                      in_=zt[:, :])

            def ad_rhs1(b, lp, lps):
                xtl = lp.tile([P, P], fp32, tag="xtl")
                nc.sync.dma_start(out=xtl[:], in_=xTloc_d[:, b * P:(b + 1) * P])
                xtlb = lp.tile([P, P], bf16, tag="xtlb")
                nc.vector.tensor_copy(xtlb[:], xtl[:])
                pad = lps.tile([P, H], fp32, tag="pad", bufs=1)
                nc.tensor.matmul(pad[:], xtlb[:], W1ext[:, C1 + H:C1 + 2 * H],
                                 start=True, stop=True)
                adb = lp.tile([P, H], bf16, tag="adb")
                nc.scalar.copy(adb[:], pad[:])
                return adb[:]

            with tc.tile_pool(name="l1", bufs=2) as lp, \
                 tc.tile_pool(name="l1ps", bufs=2, space="PSUM") as lps:
                layer_blocks(lp, lps, h1ext, H, C1, B1, bias1bc[:],
                             ROW1, C1 // 2, ad_rhs1, emit1, gbufs=3)

            # ---- AllGather z1 ----
            nc.gpsimd.collective_compute(
                "AllGather", mybir.AluOpType.bypass, replica_groups=groups,
                ins=[z1locT.ap()], outs=[z1gT.ap()])

            # ---- phase H2: h2ext rows for all N nodes ----
            with tc.tile_pool(name="h2", bufs=3) as h2p, \
                 tc.tile_pool(name="h2ps", bufs=4, space="PSUM") as h2ps:
                for k in range(cfg.NCORES):
                    # one big slab load per (remote core, k-chunk): [P, NBLK*P]
                    slabs = []
                    for kc in range(KC2):
                        sl = h2p.tile([P, NBLK * P], bf16, tag=f"slab{kc}", bufs=2)
                        nc.sync.dma_start(out=sl[:], in_=z1gT[k, kc, :, :])
                        slabs.append(sl)
                    G2 = 4
                    for b0 in range(0, NBLK, G2):
                        nb = min(G2, NBLK - b0)
                        row2 = h2p.tile([P, G2, ROW2], bf16, tag="row2", bufs=3)
                        row2f = row2[:].bitcast(fp32)
                        nc.vector.memset(row2[:].rearrange("p g r -> p (g r)"), 0)
                        for gi in range(nb):
                            b = b0 + gi
                            w = min(P, NLOC - b * P)
                            ph = h2ps.tile([P, OD + 2], fp32)
                            for kc in range(KC2):
                                nc.tensor.matmul(ph[0:w, :],
                                                 slabs[kc][:, b * P:(b + 1) * P][:, 0:w],
                                                 W2ext[:, kc, :],
                                                 start=(kc == 0), stop=(kc == KC2 - 1))
                            nc.scalar.copy(row2[0:w, gi, 0:OD], ph[0:w, 0:OD])
                            nc.vector.tensor_copy(row2f[0:w, gi, OD // 2:OD // 2 + 2],
                                                  ph[0:w, OD:OD + 2])
                        g0 = k * NLOC + b0 * P
                        gw = min(G2 * P, NLOC - b0 * P)
                        full = (gw // P) * P
                        if full > 0:
                            nc.sync.dma_start(
                                out=h2ext[g0:g0 + full, :].rearrange(
                                    "(g p) r -> p g r", p=P),
                                in_=row2[:, 0:full // P, :])
                        if gw > full:
                            nc.sync.dma_start(
                                out=h2ext[g0 + full:g0 + gw, :],
                                in_=row2[0:gw - full, full // P, :])

            # ---- L2 + pool ----
            ppool = None

            def emit2(b, rows, zf, lp, lps):
                nonlocal ppool
                pb = lp.tile([P, OD + 1], bf16, tag="pb")
                nc.scalar.copy(pb[:, 0:OD], zf[:])
                nc.vector.tensor_copy(pb[:, OD:OD + 1], onescol[:])
                sbg = lp.tile([P, G], bf16, tag="sbg")
                nc.vector.tensor_tensor(out=sbg[:],
                                        in0=batchw[:, b:b + 1].to_broadcast([P, G]),
                                        in1=giota[:], op=OP.is_equal)
                nc.tensor.matmul(ppool[:], sbg[:], pb[:],
                                 start=(b == 0), stop=(b == NBLK - 1))

            def ad_rhs2(b, lp, lps):
                pad = lps.tile([P, 1], fp32, tag="pad", bufs=1)
                for kc in range(KC2):
                    zl = lp.tile([P, P], bf16, tag="zl")
                    nc.sync.dma_start(out=zl[:], in_=z1locT[kc, :, b * P:(b + 1) * P])
                    nc.tensor.matmul(pad[:], zl[:], W2ext[:, kc, OD + 1:OD + 2],
                                     start=(kc == 0), stop=(kc == KC2 - 1))
                adb = lp.tile([P, 1], bf16, tag="adb")
                nc.scalar.copy(adb[:], pad[:])
                return adb[:]

            with tc.tile_pool(name="l2", bufs=2) as lp2, \
                 tc.tile_pool(name="l2ps", bufs=2, space="PSUM") as lps2:
                ppool = lps2.tile([G, OD + 1], fp32, tag="ppool")
                layer_blocks(lp2, lps2, h2ext, 1, OD, B2, bias2bc[:],
                             ROW2, OD // 2, ad_rhs2, emit2, gbufs=3)
                pls = lp2.tile([G, OD + 1], fp32, tag="pls")
                nc.vector.tensor_copy(pls[:], ppool[:])
                nc.sync.dma_start(out=poolpart[:, :], in_=pls[:])

            # ---- AllReduce pool, final mean ----
            nc.gpsimd.collective_compute(
                "AllReduce", mybir.AluOpType.add, replica_groups=groups,
                ins=[poolpart.ap()], outs=[poolsum.ap()])
            fin = singles.tile([G, OD + 1], fp32)
            nc.sync.dma_start(out=fin[:], in_=poolsum[:, :])
            cnt = singles.tile([G, 1], fp32)
            nc.vector.tensor_scalar(out=cnt[:], in0=fin[:, OD:OD + 1], scalar1=1.0,
                                    scalar2=None, op0=OP.max)
            rcnt = singles.tile([G, 1], fp32)
            nc.vector.reciprocal(rcnt[:], cnt[:])
            outs = singles.tile([G, OD], fp32)
            nc.scalar.activation(outs[:], fin[:, 0:OD], AF.Copy, scale=rcnt[:])
            nc.sync.dma_start(out=out_d[:, :], in_=outs[:])

    nc.compile()
    return nc


# ---------------- in_maps assembly ----------------------------------------
def _in_maps(cfg, per_core, inputs):
    xT = np.ascontiguousarray(np.asarray(inputs["x"], np.float32).T)
    KC2 = cfg.KC2
    W2 = np.asarray(inputs["W2"], np.float32).reshape(KC2, P, cfg.OUT_DIM)
    W2s = np.ascontiguousarray(W2.transpose(1, 0, 2).reshape(P, KC2 * cfg.OUT_DIM))
    common = dict(
        xT=xT,
        W1=np.asarray(inputs["W1"], np.float32),
        as1=np.asarray(inputs["att_src1"], np.float32).reshape(1, -1),
        ad1=np.asarray(inputs["att_dst1"], np.float32).reshape(1, -1),
        le1=np.asarray(inputs["lin_edge1"], np.float32),
        ae1=np.asarray(inputs["att_edge1"], np.float32).reshape(1, -1),
        b1=np.asarray(inputs["bias1"], np.float32).reshape(1, -1),
        W2=W2s,
        as2=np.asarray(inputs["att_src2"], np.float32).reshape(1, -1),
        ad2=np.asarray(inputs["att_dst2"], np.float32).reshape(1, -1),
        le2=np.asarray(inputs["lin_edge2"], np.float32),
        ae2=np.asarray(inputs["att_edge2"], np.float32).reshape(1, -1),
        b2=np.asarray(inputs["bias2"], np.float32).reshape(1, -1),
    )
    maps = []
    NBP = cfg.NBLK * P
    for k in range(cfg.NCORES):
        pc = per_core[k]
        m = dict(common)
        xTloc = np.zeros((cfg.NODE_IN, NBP), np.float32)
        xTloc[:, :cfg.NLOC] = xT[:, k * cfg.NLOC:(k + 1) * cfg.NLOC]
        m.update(srcw=pc["srcw"], dstw=pc["dstw"], xTloc=xTloc, dmod=pc["dmod"],
                 eaT=pc["eaT"], batchw=pc["batchw"])
        maps.append(m)
    return maps


_CACHE = {}


def run(cfg, inputs, trace=False):
    from concourse.bass_utils import run_bass_kernel_spmd
    per_core, M, SLOTS, EPAD = _prep(cfg, inputs["edge_index"], inputs["edge_attr"],
                                     inputs["batch"])
    key = (cfg.N, cfg.E, M)
    if key not in _CACHE:
        _CACHE[key] = _build(cfg, M)
    nc = _CACHE[key]
    maps = _in_maps(cfg, per_core, inputs)
    res = run_bass_kernel_spmd(nc, maps, core_ids=list(range(cfg.NCORES)),
                               trace=trace)
    return res


def kernel(**inputs):
    cfg = Cfg()
    res = run(cfg, inputs)
    return np.asarray(res.results[0]["out"], np.float32)
